# revision 6
# baseline (speedup 1.0000x reference)
"""Sliding-window attention (L=4096, H=2048, 16 heads, window 1024) on 8 TRN2 cores.

Collective-free sequence sharding: core c owns query rows [512c, 512c+512).
Each core receives hs rows [512c-1024, 512c+512) (zero-padded for cores 0/1)
and computes K/V projections for that window itself, so no cross-core
communication is needed. Heads are processed in a 16-iteration streaming loop
(weight columns streamed per head) to bound SBUF. Boundary masking is done
with a per-core additive bias table folded into the softmax exp.

All matmuls run in bf16 (fp32 PSUM accumulation).
"""

import sys

import numpy as np

if "/opt/trn_rl_repo" not in sys.path:
    sys.path.insert(0, "/opt/trn_rl_repo")

L = 4096
H = 2048
NH = 16
D = 128
WIN = 1024
NCORES = 8
QROWS = L // NCORES          # 512 query rows per core
WROWS = QROWS + WIN          # 1536 window rows per core
NQT = QROWS // 128           # 4 q tiles per core
NWT = WROWS // 128           # 12 window k tiles per core
NKT = 9                      # k tiles attended per q tile
ROPE_THETA = 10000.0
SCALE = float(D) ** -0.5
NEG = -1e30

_CACHE = {}


def _trace(tc, aps):
    from contextlib import ExitStack

    from concourse import mybir

    nc = tc.nc
    f32 = mybir.dt.float32
    bf16 = mybir.dt.bfloat16
    AF = mybir.ActivationFunctionType
    hsw, wq, wk, wv, wo, cosw, sinw, tsel, maskl, maskd, idf, idb, out = aps

    ctx = ExitStack()
    const = ctx.enter_context(tc.tile_pool(name="const", bufs=1))
    hstp = ctx.enter_context(tc.tile_pool(name="hst", bufs=1))
    otp = ctx.enter_context(tc.tile_pool(name="otp", bufs=1))
    wstr = ctx.enter_context(tc.tile_pool(name="wstr", bufs=2))
    kvp = ctx.enter_context(tc.tile_pool(name="kvp", bufs=2))
    rope = ctx.enter_context(tc.tile_pool(name="rope", bufs=3))
    attn = ctx.enter_context(tc.tile_pool(name="attn", bufs=3))
    phc = ctx.enter_context(tc.tile_pool(name="phc", bufs=2))
    dram = ctx.enter_context(tc.tile_pool(name="dram", bufs=1, space="DRAM"))
    ps_s = ctx.enter_context(tc.tile_pool(name="ps_s", bufs=5, space="PSUM"))
    ps_b = ctx.enter_context(tc.tile_pool(name="ps_b", bufs=3, space="PSUM"))

    # ---- constants ----
    maskl_sb = const.tile([128, 128], f32, name="maskl_sb")
    nc.sync.dma_start(out=maskl_sb, in_=maskl)
    maskd_sb = const.tile([128, 128], f32, name="maskd_sb")
    nc.sync.dma_start(out=maskd_sb, in_=maskd)
    idf_sb = const.tile([128, 128], f32, name="idf_sb")
    nc.sync.dma_start(out=idf_sb, in_=idf)
    idb_sb = const.tile([128, 128], bf16, name="idb_sb")
    nc.sync.dma_start(out=idb_sb, in_=idb)
    cos_sb = const.tile([128, WROWS], bf16, name="cos_sb")
    nc.sync.dma_start(out=cos_sb, in_=cosw)
    sin_sb = const.tile([128, WROWS], bf16, name="sin_sb")
    nc.sync.dma_start(out=sin_sb, in_=sinw)
    tsel_sb = const.tile([128, NQT, NKT], f32, name="tsel_sb")
    nc.sync.dma_start(out=tsel_sb, in_=tsel)

    # wo: one-time f32 -> bf16 cast into a DRAM bounce, streamed during o_proj
    wo_bf = dram.tile([H, H], bf16, name="wo_bf")
    nc.gpsimd.dma_start(out=wo_bf, in_=wo)

    # attention outputs, transposed: [feat-part, kt(=head), q-tile, row]
    ot_sb = otp.tile([128, 16, NQT, 128], bf16, name="ot_sb")

    # ---- load hs window and transpose to hsT [k-part, kt, row] bf16 ----
    hsT = hstp.tile([128, 16, WROWS], bf16, name="hsT")
    with tc.tile_pool(name="hsl", bufs=6) as hsl:
        for rt in range(NWT):
            hrow = []
            for half in range(2):
                hp = hsl.tile([128, 1024], f32, tag="hrow")
                nc.sync.dma_start(
                    out=hp,
                    in_=hsw[rt * 128:(rt + 1) * 128,
                            half * 1024:(half + 1) * 1024],
                )
                hrow.append(hp)
            for kt in range(16):
                tp = ps_s.tile([128, 128], f32, tag="s", name=f"htp{rt}_{kt}")
                nc.tensor.transpose(
                    tp, hrow[kt // 8][:, (kt % 8) * 128:(kt % 8 + 1) * 128],
                    idf_sb,
                )
                nc.scalar.copy(hsT[:, kt, rt * 128:(rt + 1) * 128], tp)

    def rope_pair(dst, src_ps, c0):
        """RoPE: dst[d, r] = src[d, r]*cos[d, c0+r] + src[(d+64)%128, r]*sin[d, c0+r].
        dst/src are [128, 512]; sin carries the sign for the lower half."""
        cols = slice(c0, c0 + 512)
        qbf = rope.tile([128, 512], bf16, tag="qbf")
        nc.scalar.copy(qbf, src_ps)
        qsw = rope.tile([128, 512], bf16, tag="qsw")
        nc.sync.dma_start(out=qsw[0:64, :], in_=qbf[64:128, :])
        nc.sync.dma_start(out=qsw[64:128, :], in_=qbf[0:64, :])
        t1 = rope.tile([128, 512], bf16, tag="t1")
        nc.vector.tensor_mul(t1, qbf, cos_sb[:, cols])
        t2 = rope.tile([128, 512], bf16, tag="t2")
        nc.vector.tensor_mul(t2, qsw, sin_sb[:, cols])
        nc.vector.tensor_add(dst, t1, t2)

    # ---- per-head stream: projections + RoPE + attention ----
    for h in range(NH):
        # stream this head's weight columns, casting f32 -> bf16 in the DMA
        wq_h = wstr.tile([128, 16, 128], bf16, tag="wq_h")
        nc.gpsimd.dma_start(
            out=wq_h,
            in_=wq[:, h * 128:(h + 1) * 128].rearrange("(kt p) f -> p kt f", p=128),
        )
        wk_h = wstr.tile([128, 16, 128], bf16, tag="wk_h")
        nc.gpsimd.dma_start(
            out=wk_h,
            in_=wk[:, h * 128:(h + 1) * 128].rearrange("(kt p) f -> p kt f", p=128),
        )
        wv_h = wstr.tile([128, 16, 128], bf16, tag="wv_h")
        nc.gpsimd.dma_start(
            out=wv_h,
            in_=wv[:, h * 128:(h + 1) * 128].rearrange("(kt p) f -> p kt f", p=128),
        )

        # kT for the full window (3 chunks of 512 rows), with RoPE
        kr_h = kvp.tile([128, NWT, 128], bf16, tag="kr_h")
        for rb in range(3):
            ps = ps_b.tile([128, 512], f32, tag="b", name=f"kp{h}_{rb}")
            for kt in range(16):
                nc.tensor.matmul(
                    ps,
                    lhsT=wk_h[:, kt, :],
                    rhs=hsT[:, kt, rb * 512:(rb + 1) * 512],
                    start=(kt == 0),
                    stop=(kt == 15),
                )
            dst = kr_h[:, rb * 4:(rb + 1) * 4, :].rearrange("p a b -> p (a b)")
            rope_pair(dst, ps, rb * 512)

        # qT for the core's own rows (= window rows [1024, 1536)), with RoPE
        qr_h = kvp.tile([128, NQT, 128], bf16, tag="qr_h")
        ps = ps_b.tile([128, 512], f32, tag="b", name=f"qp{h}")
        for kt in range(16):
            nc.tensor.matmul(
                ps,
                lhsT=wq_h[:, kt, :],
                rhs=hsT[:, kt, 1024:1536],
                start=(kt == 0),
                stop=(kt == 15),
            )
        rope_pair(qr_h.rearrange("p a b -> p (a b)"), ps, 1024)

        # vT for the window, then PE-transpose to natural V (+ ones column)
        v_h = kvp.tile([128, NWT, 130], bf16, tag="v_h")
        nc.vector.memset(v_h[:, :, 128:130], 0.0)
        nc.vector.memset(v_h[:, :, 128:129], 1.0)
        for rb in range(3):
            ps = ps_b.tile([128, 512], f32, tag="b", name=f"vp{h}_{rb}")
            for kt in range(16):
                nc.tensor.matmul(
                    ps,
                    lhsT=wv_h[:, kt, :],
                    rhs=hsT[:, kt, rb * 512:(rb + 1) * 512],
                    start=(kt == 0),
                    stop=(kt == 15),
                )
            vt_bf = rope.tile([128, 512], bf16, tag="vt_bf")
            nc.scalar.copy(vt_bf, ps)
            for j in range(4):
                tp = ps_s.tile([128, 128], bf16, tag="s", name=f"vt{h}_{rb}_{j}")
                nc.tensor.transpose(tp, vt_bf[:, j * 128:(j + 1) * 128], idb_sb)
                nc.scalar.copy(v_h[:, rb * 4 + j, 0:128], tp)

        # attention for the 4 local q tiles
        for lq in range(NQT):
            o_ps = ps_s.tile([128, 132], f32, tag="s", name=f"o{h}_{lq}")
            p_sb = attn.tile([128, NKT, 128], bf16, tag="p_sb")
            for t in range(NKT):
                st_ps = ps_s.tile([128, 128], f32, tag="s", name=f"st{h}_{lq}_{t}")
                nc.tensor.matmul(
                    st_ps,
                    lhsT=kr_h[:, lq + t, :],
                    rhs=qr_h[:, lq, :],
                    start=True,
                    stop=True,
                )
                if t == NKT - 1:
                    nc.vector.tensor_add(st_ps, st_ps, maskd_sb)
                elif t == 0:
                    nc.vector.tensor_add(st_ps, st_ps, maskl_sb)
                nc.scalar.activation(
                    p_sb[:, t, :], st_ps, AF.Exp,
                    bias=tsel_sb[:, lq, t:t + 1], scale=SCALE,
                )
            for t in range(NKT):
                nc.tensor.matmul(
                    o_ps[:, 0:129],
                    lhsT=p_sb[:, t, :],
                    rhs=v_h[:, lq + t, 0:129],
                    start=(t == 0),
                    stop=(t == NKT - 1),
                )
            rinv = attn.tile([128, 1], f32, tag="rinv")
            nc.vector.reciprocal(rinv, o_ps[:, 128:129])
            ao = attn.tile([128, 128], bf16, tag="ao")
            nc.vector.tensor_scalar_mul(ao, o_ps[:, 0:128], rinv)
            tp = ps_s.tile([128, 128], bf16, tag="s", name=f"aot{h}_{lq}")
            nc.tensor.transpose(tp, ao, idb_sb)
            nc.scalar.copy(ot_sb[:, h, lq, :], tp)

    # ---- o_proj: out[rows, :] = ot.T @ wo, streaming wo (bf16 bounce).
    # Each streamed wo tile is reused for a pair of q-tiles to halve traffic.
    for lqp in range(NQT // 2):
        for nb in range(4):
            pss = [
                ps_b.tile([128, 512], f32, tag="b", name=f"op{lqp}_{nb}_{i}")
                for i in range(2)
            ]
            for kt in range(16):
                wos = phc.tile([128, 512], bf16, tag="wos", bufs=6)
                nc.sync.dma_start(
                    out=wos,
                    in_=wo_bf[kt * 128:(kt + 1) * 128, nb * 512:(nb + 1) * 512],
                )
                for i in range(2):
                    nc.tensor.matmul(
                        pss[i], lhsT=ot_sb[:, kt, lqp * 2 + i, :], rhs=wos,
                        start=(kt == 0), stop=(kt == 15),
                    )
            for i in range(2):
                lq = lqp * 2 + i
                ob = phc.tile([128, 512], f32, tag="ob", bufs=3)
                nc.scalar.copy(ob, pss[i])
                nc.sync.dma_start(
                    out=out[lq, :, nb * 512:(nb + 1) * 512], in_=ob
                )

    ctx.close()


def _build():
    import concourse.bacc as bacc
    import concourse.tile as tile
    from concourse import mybir

    f32 = mybir.dt.float32
    bf16 = mybir.dt.bfloat16

    nc = bacc.Bacc("TRN2", target_bir_lowering=False, debug=False,
                   num_devices=NCORES)
    aps = [
        nc.dram_tensor("hsw", [WROWS, H], f32, kind="ExternalInput").ap(),
        nc.dram_tensor("wq", [H, H], f32, kind="ExternalInput").ap(),
        nc.dram_tensor("wk", [H, H], f32, kind="ExternalInput").ap(),
        nc.dram_tensor("wv", [H, H], f32, kind="ExternalInput").ap(),
        nc.dram_tensor("wo", [H, H], f32, kind="ExternalInput").ap(),
        nc.dram_tensor("cosw", [D, WROWS], bf16, kind="ExternalInput").ap(),
        nc.dram_tensor("sinw", [D, WROWS], bf16, kind="ExternalInput").ap(),
        nc.dram_tensor("tsel", [D, NQT, NKT], f32, kind="ExternalInput").ap(),
        nc.dram_tensor("maskl", [128, 128], f32, kind="ExternalInput").ap(),
        nc.dram_tensor("maskd", [128, 128], f32, kind="ExternalInput").ap(),
        nc.dram_tensor("idf", [128, 128], f32, kind="ExternalInput").ap(),
        nc.dram_tensor("idb", [128, 128], bf16, kind="ExternalInput").ap(),
        nc.dram_tensor("out", [NQT, 128, H], f32, kind="ExternalOutput").ap(),
    ]
    with tile.TileContext(nc) as tc:
        _trace(tc, aps)
    nc.compile()
    return nc


def _host_constants():
    import ml_dtypes

    inv = 1.0 / (ROPE_THETA ** (np.arange(0, D, 2, dtype=np.float64) / D))
    ii = np.arange(128)
    # masks for S^T [j, i] tiles; valid -> 0, invalid -> NEG
    maskl = np.where(ii[:, None] > ii[None, :], 0.0, NEG).astype(np.float32)
    maskd = np.where(ii[:, None] <= ii[None, :], 0.0, NEG).astype(np.float32)
    idf = np.eye(128, dtype=np.float32)
    idb = np.eye(128).astype(ml_dtypes.bfloat16)

    cos_list, sin_list, tsel_list = [], [], []
    for c in range(NCORES):
        # window rows are global positions [512c - 1024, 512c + 512)
        pos = np.arange(c * QROWS - WIN, c * QROWS + QROWS, dtype=np.float64)
        pos = np.maximum(pos, 0.0)         # pad rows: value irrelevant (masked)
        ang = inv[:, None] * pos[None, :]  # [64, WROWS]
        cos_list.append(np.concatenate([np.cos(ang), np.cos(ang)], 0)
                        .astype(ml_dtypes.bfloat16))
        sin_list.append(np.concatenate([-np.sin(ang), np.sin(ang)], 0)
                        .astype(ml_dtypes.bfloat16))
        # tsel[lq, t] = 0 if local k-tile lq+t is a real (non-pad) tile else NEG
        ts = np.zeros((NQT, NKT), np.float32)
        for lq in range(NQT):
            for t in range(NKT):
                gkt = (c * QROWS - WIN) // 128 + lq + t
                if gkt < 0:
                    ts[lq, t] = NEG
        tsel_list.append(np.broadcast_to(ts, (128, NQT, NKT)).copy())
    return cos_list, sin_list, tsel_list, maskl, maskd, idf, idb


def _get_state():
    if "nc" not in _CACHE:
        _CACHE["nc"] = _build()
        _CACHE["consts"] = _host_constants()
    return _CACHE["nc"], _CACHE["consts"]


def _in_maps(hidden_states, wq, wk, wv, wo, consts):
    hs = np.ascontiguousarray(np.asarray(hidden_states, np.float32).reshape(L, H))
    wq = np.ascontiguousarray(np.asarray(wq, np.float32))
    wk = np.ascontiguousarray(np.asarray(wk, np.float32))
    wv = np.ascontiguousarray(np.asarray(wv, np.float32))
    wo = np.ascontiguousarray(np.asarray(wo, np.float32))
    cos_list, sin_list, tsel_list, maskl, maskd, idf, idb = consts
    maps = []
    for c in range(NCORES):
        lo = c * QROWS - WIN
        hsw = np.zeros((WROWS, H), np.float32)
        src_lo = max(lo, 0)
        hsw[src_lo - lo:] = hs[src_lo:c * QROWS + QROWS]
        maps.append({
            "hsw": hsw,
            "wq": wq,
            "wk": wk,
            "wv": wv,
            "wo": wo,
            "cosw": cos_list[c],
            "sinw": sin_list[c],
            "tsel": tsel_list[c],
            "maskl": maskl,
            "maskd": maskd,
            "idf": idf,
            "idb": idb,
        })
    return maps


def _gather(results):
    full = np.empty((L, H), np.float32)
    for c in range(NCORES):
        full[c * QROWS:(c + 1) * QROWS] = results[c]["out"].reshape(QROWS, H)
    return full.reshape(1, L, H)


def kernel(hidden_states, wq, wk, wv, wo):
    from concourse.bass_utils import run_bass_kernel_spmd

    nc, consts = _get_state()
    maps = _in_maps(hidden_states, wq, wk, wv, wo, consts)
    res = run_bass_kernel_spmd(nc, maps, core_ids=list(range(NCORES)))
    return _gather(res.results)


# revision 9
# speedup vs baseline: 284.8217x; 284.8217x over previous
"""Sliding-window attention (L=4096, H=2048, 16 heads, window 1024) on 8 TRN2 cores.

Collective-free sequence sharding: core c owns query rows [512c, 512c+512).
Each core receives hs rows [512c-1024, 512c+512) (zero-padded for cores 0/1)
and computes K/V projections for that window itself, so no cross-core
communication is needed. Heads are processed in a 16-iteration streaming loop
(weight columns streamed per head) to bound SBUF. Boundary masking is done
with a per-core additive bias table folded into the softmax exp.

All matmuls run in bf16 (fp32 PSUM accumulation).
"""

import sys

import numpy as np

if "/opt/trn_rl_repo" not in sys.path:
    sys.path.insert(0, "/opt/trn_rl_repo")

L = 4096
H = 2048
NH = 16
D = 128
WIN = 1024
NCORES = 8
QROWS = L // NCORES          # 512 query rows per core
WROWS = QROWS + WIN          # 1536 window rows per core
NQT = QROWS // 128           # 4 q tiles per core
NWT = WROWS // 128           # 12 window k tiles per core
NKT = 9                      # k tiles attended per q tile
ROPE_THETA = 10000.0
SCALE = float(D) ** -0.5
NEG = -1e30

_CACHE = {}


def _trace(tc, aps):
    from contextlib import ExitStack

    from concourse import mybir

    nc = tc.nc
    f32 = mybir.dt.float32
    bf16 = mybir.dt.bfloat16
    AF = mybir.ActivationFunctionType
    hsw, wq, wk, wv, wo, cosw, sinw, tsel, maskl, maskd, idf, idb, out = aps

    ctx = ExitStack()
    const = ctx.enter_context(tc.tile_pool(name="const", bufs=1))
    hstp = ctx.enter_context(tc.tile_pool(name="hst", bufs=1))
    otp = ctx.enter_context(tc.tile_pool(name="otp", bufs=1))
    wstr = ctx.enter_context(tc.tile_pool(name="wstr", bufs=2))
    kvp = ctx.enter_context(tc.tile_pool(name="kvp", bufs=2))
    rope = ctx.enter_context(tc.tile_pool(name="rope", bufs=3))
    attn = ctx.enter_context(tc.tile_pool(name="attn", bufs=3))
    phc = ctx.enter_context(tc.tile_pool(name="phc", bufs=2))
    dram = ctx.enter_context(tc.tile_pool(name="dram", bufs=1, space="DRAM"))
    ps_s = ctx.enter_context(tc.tile_pool(name="ps_s", bufs=5, space="PSUM"))
    ps_b = ctx.enter_context(tc.tile_pool(name="ps_b", bufs=3, space="PSUM"))

    # ---- constants ----
    maskl_sb = const.tile([128, 128], f32, name="maskl_sb")
    nc.sync.dma_start(out=maskl_sb, in_=maskl)
    maskd_sb = const.tile([128, 128], f32, name="maskd_sb")
    nc.sync.dma_start(out=maskd_sb, in_=maskd)
    idf_sb = const.tile([128, 128], f32, name="idf_sb")
    nc.sync.dma_start(out=idf_sb, in_=idf)
    idb_sb = const.tile([128, 128], bf16, name="idb_sb")
    nc.sync.dma_start(out=idb_sb, in_=idb)
    cos_sb = const.tile([128, WROWS], bf16, name="cos_sb")
    nc.sync.dma_start(out=cos_sb, in_=cosw)
    sin_sb = const.tile([128, WROWS], bf16, name="sin_sb")
    nc.sync.dma_start(out=sin_sb, in_=sinw)
    tsel_sb = const.tile([128, NQT, NKT], f32, name="tsel_sb")
    nc.sync.dma_start(out=tsel_sb, in_=tsel)

    # wo: one-time f32 -> bf16 cast into a DRAM bounce, streamed during o_proj
    wo_bf = dram.tile([H, H], bf16, name="wo_bf")
    nc.gpsimd.dma_start(out=wo_bf, in_=wo)

    # attention outputs, transposed: [feat-part, kt(=head), q-tile, row]
    ot_sb = otp.tile([128, 16, NQT, 128], bf16, name="ot_sb")

    # ---- load hs window (cast to bf16 in-DMA), transpose to hsT ----
    hsT = hstp.tile([128, 16, WROWS], bf16, name="hsT")
    with tc.tile_pool(name="hsl", bufs=6) as hsl:
        for rt in range(NWT):
            hrow = []
            for half in range(2):
                hp = hsl.tile([128, 1024], bf16, tag="hrow")
                nc.gpsimd.dma_start(
                    out=hp,
                    in_=hsw[rt * 128:(rt + 1) * 128,
                            half * 1024:(half + 1) * 1024],
                )
                hrow.append(hp)
            for kt in range(16):
                tp = ps_s.tile([128, 128], bf16, tag="s", name=f"htp{rt}_{kt}")
                nc.tensor.transpose(
                    tp, hrow[kt // 8][:, (kt % 8) * 128:(kt % 8 + 1) * 128],
                    idb_sb,
                )
                nc.scalar.copy(hsT[:, kt, rt * 128:(rt + 1) * 128], tp)

    def rope_pair(dst, src_ps, c0):
        """RoPE: dst[d, r] = src[d, r]*cos[d, c0+r] + src[(d+64)%128, r]*sin[d, c0+r].
        dst/src are [128, 512]; sin carries the sign for the lower half."""
        cols = slice(c0, c0 + 512)
        qbf = rope.tile([128, 512], bf16, tag="qbf")
        nc.scalar.copy(qbf, src_ps)
        qsw = rope.tile([128, 512], bf16, tag="qsw")
        nc.sync.dma_start(out=qsw[0:64, :], in_=qbf[64:128, :])
        nc.sync.dma_start(out=qsw[64:128, :], in_=qbf[0:64, :])
        t1 = rope.tile([128, 512], bf16, tag="t1")
        nc.vector.tensor_mul(t1, qbf, cos_sb[:, cols])
        t2 = rope.tile([128, 512], bf16, tag="t2")
        nc.vector.tensor_mul(t2, qsw, sin_sb[:, cols])
        nc.vector.tensor_add(dst, t1, t2)

    # ---- per-head stream: projections + RoPE + attention ----
    for h in range(NH):
        # stream this head's weight columns, casting f32 -> bf16 in the DMA
        wq_h = wstr.tile([128, 16, 128], bf16, tag="wq_h")
        nc.gpsimd.dma_start(
            out=wq_h,
            in_=wq[:, h * 128:(h + 1) * 128].rearrange("(kt p) f -> p kt f", p=128),
        )
        wk_h = wstr.tile([128, 16, 128], bf16, tag="wk_h")
        nc.gpsimd.dma_start(
            out=wk_h,
            in_=wk[:, h * 128:(h + 1) * 128].rearrange("(kt p) f -> p kt f", p=128),
        )
        wv_h = wstr.tile([128, 16, 128], bf16, tag="wv_h")
        nc.gpsimd.dma_start(
            out=wv_h,
            in_=wv[:, h * 128:(h + 1) * 128].rearrange("(kt p) f -> p kt f", p=128),
        )

        # kT for the full window (3 chunks of 512 rows), with RoPE
        kr_h = kvp.tile([128, NWT, 128], bf16, tag="kr_h")
        for rb in range(3):
            ps = ps_b.tile([128, 512], f32, tag="b", name=f"kp{h}_{rb}")
            for kt in range(16):
                nc.tensor.matmul(
                    ps,
                    lhsT=wk_h[:, kt, :],
                    rhs=hsT[:, kt, rb * 512:(rb + 1) * 512],
                    start=(kt == 0),
                    stop=(kt == 15),
                )
            dst = kr_h[:, rb * 4:(rb + 1) * 4, :].rearrange("p a b -> p (a b)")
            rope_pair(dst, ps, rb * 512)

        # qT for the core's own rows (= window rows [1024, 1536)), with RoPE
        qr_h = kvp.tile([128, NQT, 128], bf16, tag="qr_h")
        ps = ps_b.tile([128, 512], f32, tag="b", name=f"qp{h}")
        for kt in range(16):
            nc.tensor.matmul(
                ps,
                lhsT=wq_h[:, kt, :],
                rhs=hsT[:, kt, 1024:1536],
                start=(kt == 0),
                stop=(kt == 15),
            )
        rope_pair(qr_h.rearrange("p a b -> p (a b)"), ps, 1024)

        # vT for the window, then PE-transpose to natural V (+ ones column)
        v_h = kvp.tile([128, NWT, 130], bf16, tag="v_h")
        nc.vector.memset(v_h[:, :, 128:130], 0.0)
        nc.vector.memset(v_h[:, :, 128:129], 1.0)
        for rb in range(3):
            ps = ps_b.tile([128, 512], f32, tag="b", name=f"vp{h}_{rb}")
            for kt in range(16):
                nc.tensor.matmul(
                    ps,
                    lhsT=wv_h[:, kt, :],
                    rhs=hsT[:, kt, rb * 512:(rb + 1) * 512],
                    start=(kt == 0),
                    stop=(kt == 15),
                )
            vt_bf = rope.tile([128, 512], bf16, tag="vt_bf")
            nc.scalar.copy(vt_bf, ps)
            for j in range(4):
                tp = ps_s.tile([128, 128], bf16, tag="s", name=f"vt{h}_{rb}_{j}")
                nc.tensor.transpose(tp, vt_bf[:, j * 128:(j + 1) * 128], idb_sb)
                nc.scalar.copy(v_h[:, rb * 4 + j, 0:128], tp)

        # attention for the 4 local q tiles
        for lq in range(NQT):
            o_ps = ps_s.tile([128, 132], f32, tag="s", name=f"o{h}_{lq}")
            p_sb = attn.tile([128, NKT, 128], bf16, tag="p_sb")
            for t in range(NKT):
                st_ps = ps_s.tile([128, 128], f32, tag="s", name=f"st{h}_{lq}_{t}")
                nc.tensor.matmul(
                    st_ps,
                    lhsT=kr_h[:, lq + t, :],
                    rhs=qr_h[:, lq, :],
                    start=True,
                    stop=True,
                )
                if t == NKT - 1:
                    nc.vector.tensor_add(st_ps, st_ps, maskd_sb)
                elif t == 0:
                    nc.vector.tensor_add(st_ps, st_ps, maskl_sb)
                nc.scalar.activation(
                    p_sb[:, t, :], st_ps, AF.Exp,
                    bias=tsel_sb[:, lq, t:t + 1], scale=SCALE,
                )
            for t in range(NKT):
                nc.tensor.matmul(
                    o_ps[:, 0:129],
                    lhsT=p_sb[:, t, :],
                    rhs=v_h[:, lq + t, 0:129],
                    start=(t == 0),
                    stop=(t == NKT - 1),
                )
            rinv = attn.tile([128, 1], f32, tag="rinv")
            nc.vector.reciprocal(rinv, o_ps[:, 128:129])
            ao = attn.tile([128, 128], bf16, tag="ao")
            nc.vector.tensor_scalar_mul(ao, o_ps[:, 0:128], rinv)
            tp = ps_s.tile([128, 128], bf16, tag="s", name=f"aot{h}_{lq}")
            nc.tensor.transpose(tp, ao, idb_sb)
            nc.scalar.copy(ot_sb[:, h, lq, :], tp)

    # ---- o_proj: out[rows, :] = ot.T @ wo, streaming wo (bf16 bounce).
    # Each streamed wo tile is reused for a pair of q-tiles to halve traffic.
    for lqp in range(NQT // 2):
        for nb in range(4):
            pss = [
                ps_b.tile([128, 512], f32, tag="b", name=f"op{lqp}_{nb}_{i}")
                for i in range(2)
            ]
            for kt in range(16):
                wos = phc.tile([128, 512], bf16, tag="wos", bufs=6)
                nc.sync.dma_start(
                    out=wos,
                    in_=wo_bf[kt * 128:(kt + 1) * 128, nb * 512:(nb + 1) * 512],
                )
                for i in range(2):
                    nc.tensor.matmul(
                        pss[i], lhsT=ot_sb[:, kt, lqp * 2 + i, :], rhs=wos,
                        start=(kt == 0), stop=(kt == 15),
                    )
            for i in range(2):
                lq = lqp * 2 + i
                ob = phc.tile([128, 512], f32, tag="ob", bufs=3)
                nc.scalar.copy(ob, pss[i])
                nc.sync.dma_start(
                    out=out[lq, :, nb * 512:(nb + 1) * 512], in_=ob
                )

    ctx.close()


def _build():
    import concourse.bacc as bacc
    import concourse.tile as tile
    from concourse import mybir

    f32 = mybir.dt.float32
    bf16 = mybir.dt.bfloat16

    nc = bacc.Bacc("TRN2", target_bir_lowering=False, debug=False,
                   num_devices=NCORES)
    aps = [
        nc.dram_tensor("hsw", [WROWS, H], f32, kind="ExternalInput").ap(),
        nc.dram_tensor("wq", [H, H], f32, kind="ExternalInput").ap(),
        nc.dram_tensor("wk", [H, H], f32, kind="ExternalInput").ap(),
        nc.dram_tensor("wv", [H, H], f32, kind="ExternalInput").ap(),
        nc.dram_tensor("wo", [H, H], f32, kind="ExternalInput").ap(),
        nc.dram_tensor("cosw", [D, WROWS], bf16, kind="ExternalInput").ap(),
        nc.dram_tensor("sinw", [D, WROWS], bf16, kind="ExternalInput").ap(),
        nc.dram_tensor("tsel", [D, NQT, NKT], f32, kind="ExternalInput").ap(),
        nc.dram_tensor("maskl", [128, 128], f32, kind="ExternalInput").ap(),
        nc.dram_tensor("maskd", [128, 128], f32, kind="ExternalInput").ap(),
        nc.dram_tensor("idf", [128, 128], f32, kind="ExternalInput").ap(),
        nc.dram_tensor("idb", [128, 128], bf16, kind="ExternalInput").ap(),
        nc.dram_tensor("out", [NQT, 128, H], f32, kind="ExternalOutput").ap(),
    ]
    with tile.TileContext(nc) as tc:
        _trace(tc, aps)
    nc.compile()
    return nc


def _host_constants():
    import ml_dtypes

    inv = 1.0 / (ROPE_THETA ** (np.arange(0, D, 2, dtype=np.float64) / D))
    ii = np.arange(128)
    # masks for S^T [j, i] tiles; valid -> 0, invalid -> NEG
    maskl = np.where(ii[:, None] > ii[None, :], 0.0, NEG).astype(np.float32)
    maskd = np.where(ii[:, None] <= ii[None, :], 0.0, NEG).astype(np.float32)
    idf = np.eye(128, dtype=np.float32)
    idb = np.eye(128).astype(ml_dtypes.bfloat16)

    cos_list, sin_list, tsel_list = [], [], []
    for c in range(NCORES):
        # window rows are global positions [512c - 1024, 512c + 512)
        pos = np.arange(c * QROWS - WIN, c * QROWS + QROWS, dtype=np.float64)
        pos = np.maximum(pos, 0.0)         # pad rows: value irrelevant (masked)
        ang = inv[:, None] * pos[None, :]  # [64, WROWS]
        cos_list.append(np.concatenate([np.cos(ang), np.cos(ang)], 0)
                        .astype(ml_dtypes.bfloat16))
        sin_list.append(np.concatenate([-np.sin(ang), np.sin(ang)], 0)
                        .astype(ml_dtypes.bfloat16))
        # tsel[lq, t] = 0 if local k-tile lq+t is a real (non-pad) tile else NEG
        ts = np.zeros((NQT, NKT), np.float32)
        for lq in range(NQT):
            for t in range(NKT):
                gkt = (c * QROWS - WIN) // 128 + lq + t
                if gkt < 0:
                    ts[lq, t] = NEG
        tsel_list.append(np.broadcast_to(ts, (128, NQT, NKT)).copy())
    return cos_list, sin_list, tsel_list, maskl, maskd, idf, idb


def _get_state():
    if "nc" not in _CACHE:
        _CACHE["nc"] = _build()
        _CACHE["consts"] = _host_constants()
    return _CACHE["nc"], _CACHE["consts"]


def _in_maps(hidden_states, wq, wk, wv, wo, consts):
    hs = np.ascontiguousarray(np.asarray(hidden_states, np.float32).reshape(L, H))
    wq = np.ascontiguousarray(np.asarray(wq, np.float32))
    wk = np.ascontiguousarray(np.asarray(wk, np.float32))
    wv = np.ascontiguousarray(np.asarray(wv, np.float32))
    wo = np.ascontiguousarray(np.asarray(wo, np.float32))
    cos_list, sin_list, tsel_list, maskl, maskd, idf, idb = consts
    maps = []
    for c in range(NCORES):
        lo = c * QROWS - WIN
        hsw = np.zeros((WROWS, H), np.float32)
        src_lo = max(lo, 0)
        hsw[src_lo - lo:] = hs[src_lo:c * QROWS + QROWS]
        maps.append({
            "hsw": hsw,
            "wq": wq,
            "wk": wk,
            "wv": wv,
            "wo": wo,
            "cosw": cos_list[c],
            "sinw": sin_list[c],
            "tsel": tsel_list[c],
            "maskl": maskl,
            "maskd": maskd,
            "idf": idf,
            "idb": idb,
        })
    return maps


def _gather(results):
    full = np.empty((L, H), np.float32)
    for c in range(NCORES):
        full[c * QROWS:(c + 1) * QROWS] = results[c]["out"].reshape(QROWS, H)
    return full.reshape(1, L, H)


class _Runner:
    """Persistent jitted shard_map executable over the 8 axon cores.

    Mirrors bass2jax.run_bass_via_pjrt's multi-core path, but builds the
    jitted callable once (so repeat kernel() calls skip retracing) and
    skips output-buffer donation (this kernel writes every output element,
    so the pre-zeroed-output contract is not needed).
    """

    def __init__(self, nc):
        import jax
        from jax.sharding import Mesh, PartitionSpec
        from jax.experimental.shard_map import shard_map
        from concourse import mybir
        from concourse import bass2jax

        bass2jax.install_neuronx_cc_hook()

        partition_name = (
            nc.partition_id_tensor.name if nc.partition_id_tensor else None
        )
        in_names, out_names, out_avals, zero_outs = [], [], [], []
        for alloc in nc.m.functions[0].allocations:
            if not isinstance(alloc, mybir.MemoryLocationSet):
                continue
            name = alloc.memorylocations[0].name
            if alloc.kind == "ExternalInput":
                if name != partition_name:
                    in_names.append(name)
            elif alloc.kind == "ExternalOutput":
                out_names.append(name)
                shape = tuple(alloc.tensor_shape)
                dtype = mybir.dt.np(alloc.dtype)
                out_avals.append(jax.core.ShapedArray(shape, dtype))
                zero_outs.append(np.zeros(shape, dtype))
        self.n_params = len(in_names)
        self.in_names = list(in_names)
        self.out_names = out_names
        all_names = in_names + out_names
        if partition_name is not None:
            all_names = all_names + [partition_name]

        def _body(*args):
            operands = list(args)
            if partition_name is not None:
                operands.append(bass2jax.partition_id_tensor())
            outs = bass2jax._bass_exec_p.bind(
                *operands,
                out_avals=tuple(out_avals),
                in_names=tuple(all_names),
                out_names=tuple(out_names),
                lowering_input_output_aliases=(),
                sim_require_finite=True,
                sim_require_nnan=True,
                nc=nc,
            )
            return tuple(outs)

        devices = jax.devices()[:NCORES]
        assert len(devices) == NCORES
        self.mesh = Mesh(np.asarray(devices), ("core",))
        in_specs = (PartitionSpec("core"),) * (self.n_params + len(out_names))
        out_specs = (PartitionSpec("core"),) * len(out_names)
        self.sharded = jax.jit(
            shard_map(_body, mesh=self.mesh, in_specs=in_specs,
                      out_specs=out_specs, check_rep=False),
            keep_unused=True,
        )
        self.out_avals = out_avals
        self.concat_zeros = [
            np.zeros((NCORES * z.shape[0], *z.shape[1:]), z.dtype)
            for z in zero_outs
        ]
        self._dev_args = None

    def pack(self, maps):
        return [
            np.concatenate([np.asarray(maps[c][n]) for c in range(NCORES)], axis=0)
            for n in self.in_names
        ]

    def run(self, maps):
        import jax

        concat_in = self.pack(maps)
        out_arrs = self.sharded(*concat_in, *self.concat_zeros)
        return [
            {
                n: np.asarray(out_arrs[i]).reshape(
                    NCORES, *self.out_avals[i].shape)[c]
                for i, n in enumerate(self.out_names)
            }
            for c in range(NCORES)
        ]

    def bench(self, maps, iters=10):
        """Time repeated executions with inputs resident on device."""
        import time

        import jax

        args = [jax.device_put(a) for a in self.pack(maps)]
        args += [jax.device_put(z) for z in self.concat_zeros]
        out = self.sharded(*args)  # warm
        jax.block_until_ready(out)
        t0 = time.perf_counter()
        for _ in range(iters):
            out = self.sharded(*args)
        jax.block_until_ready(out)
        return (time.perf_counter() - t0) / iters


def _get_runner():
    nc, consts = _get_state()
    if "runner" not in _CACHE:
        _CACHE["runner"] = _Runner(nc)
    return _CACHE["runner"], consts


def kernel(hidden_states, wq, wk, wv, wo):
    runner, consts = _get_runner()
    maps = _in_maps(hidden_states, wq, wk, wv, wo, consts)
    return _gather(runner.run(maps))


def bench(hidden_states, wq, wk, wv, wo, iters=10):
    runner, consts = _get_runner()
    maps = _in_maps(hidden_states, wq, wk, wv, wo, consts)
    return runner.bench(maps, iters=iters)


# revision 14
# speedup vs baseline: 285.8623x; 1.0037x over previous
"""Sliding-window attention (L=4096, H=2048, 16 heads, window 1024) on 8 TRN2 cores.

Collective-free sequence sharding: core c owns query rows [512c, 512c+512).
Each core receives hs rows [512c-1024, 512c+512) (zero-padded for cores 0/1)
and computes K/V projections for that window itself, so no cross-core
communication is needed. Heads are processed in a 16-iteration streaming loop
(weight columns streamed per head) to bound SBUF. Boundary masking is done
with a per-core additive bias table folded into the softmax exp.

All matmuls run in bf16 (fp32 PSUM accumulation).
"""

import sys

import numpy as np

if "/opt/trn_rl_repo" not in sys.path:
    sys.path.insert(0, "/opt/trn_rl_repo")

L = 4096
H = 2048
NH = 16
D = 128
WIN = 1024
NCORES = 8
QROWS = L // NCORES          # 512 query rows per core
WROWS = QROWS + WIN          # 1536 window rows per core
NQT = QROWS // 128           # 4 q tiles per core
NWT = WROWS // 128           # 12 window k tiles per core
NKT = 9                      # k tiles attended per q tile
ROPE_THETA = 10000.0
SCALE = float(D) ** -0.5
NEG = -1e30

_CACHE = {}


def _trace(tc, aps):
    from contextlib import ExitStack

    from concourse import mybir

    nc = tc.nc
    f32 = mybir.dt.float32
    bf16 = mybir.dt.bfloat16
    AF = mybir.ActivationFunctionType
    hsw, wq, wk, wv, wo, cosw, sinw, tsel, maskl, maskd, idf, idb, out = aps

    ctx = ExitStack()
    const = ctx.enter_context(tc.tile_pool(name="const", bufs=1))
    hstp = ctx.enter_context(tc.tile_pool(name="hst", bufs=1))
    otp = ctx.enter_context(tc.tile_pool(name="otp", bufs=1))
    wstr = ctx.enter_context(tc.tile_pool(name="wstr", bufs=2))
    kvp = ctx.enter_context(tc.tile_pool(name="kvp", bufs=2))
    rope = ctx.enter_context(tc.tile_pool(name="rope", bufs=2))
    attn = ctx.enter_context(tc.tile_pool(name="attn", bufs=3))
    phc = ctx.enter_context(tc.tile_pool(name="phc", bufs=2))
    dram = ctx.enter_context(tc.tile_pool(name="dram", bufs=1, space="DRAM"))
    ps_s = ctx.enter_context(tc.tile_pool(name="ps_s", bufs=5, space="PSUM"))
    ps_b = ctx.enter_context(tc.tile_pool(name="ps_b", bufs=3, space="PSUM"))

    # ---- constants ----
    maskl_sb = const.tile([128, 128], f32, name="maskl_sb")
    nc.sync.dma_start(out=maskl_sb, in_=maskl)
    maskd_sb = const.tile([128, 128], f32, name="maskd_sb")
    nc.sync.dma_start(out=maskd_sb, in_=maskd)
    idf_sb = const.tile([128, 128], f32, name="idf_sb")
    nc.sync.dma_start(out=idf_sb, in_=idf)
    idb_sb = const.tile([128, 128], bf16, name="idb_sb")
    nc.sync.dma_start(out=idb_sb, in_=idb)
    cos_sb = const.tile([128, WROWS], bf16, name="cos_sb")
    nc.sync.dma_start(out=cos_sb, in_=cosw)
    sin_sb = const.tile([128, WROWS], bf16, name="sin_sb")
    nc.sync.dma_start(out=sin_sb, in_=sinw)
    tsel_sb = const.tile([128, NQT, NKT], f32, name="tsel_sb")
    nc.sync.dma_start(out=tsel_sb, in_=tsel)

    # wo: one-time f32 -> bf16 cast into a DRAM bounce, streamed during o_proj.
    # HWDGE loads + DVE casts (SWDGE cast-DMA descriptor generation is slow).
    wo_bf = dram.tile([H, H], bf16, name="wo_bf")
    for i in range(32):
        wob_f = phc.tile([128, 1024], f32, tag="wob_f", bufs=1)
        nc.sync.dma_start(
            out=wob_f,
            in_=wo[(i // 2) * 128:(i // 2 + 1) * 128,
                   (i % 2) * 1024:(i % 2 + 1) * 1024],
        )
        wob_b = phc.tile([128, 1024], bf16, tag="wob_b", bufs=2)
        nc.vector.tensor_copy(wob_b, wob_f)
        nc.sync.dma_start(
            out=wo_bf[(i // 2) * 128:(i // 2 + 1) * 128,
                      (i % 2) * 1024:(i % 2 + 1) * 1024],
            in_=wob_b,
        )

    # attention outputs, transposed: [feat-part, kt(=head), q-tile, row]
    ot_sb = otp.tile([128, 16, NQT, 128], bf16, name="ot_sb")

    # ---- load hs window (cast to bf16 in-DMA), transpose to hsT ----
    hsT = hstp.tile([128, 16, WROWS], bf16, name="hsT")
    with tc.tile_pool(name="hsl", bufs=4) as hsl:
        for rt in range(NWT):
            hrow = []
            for half in range(2):
                hp = hsl.tile([128, 1024], f32, tag="hrow")
                nc.sync.dma_start(
                    out=hp,
                    in_=hsw[rt * 128:(rt + 1) * 128,
                            half * 1024:(half + 1) * 1024],
                )
                hrow.append(hp)
            for kt in range(16):
                tp = ps_s.tile([128, 128], f32, tag="s", name=f"htp{rt}_{kt}")
                nc.tensor.transpose(
                    tp, hrow[kt // 8][:, (kt % 8) * 128:(kt % 8 + 1) * 128],
                    idf_sb,
                )
                nc.scalar.copy(hsT[:, kt, rt * 128:(rt + 1) * 128], tp)

    def rope_pair(dst, src_ps, c0):
        """RoPE: dst[d, r] = src[d, r]*cos[d, c0+r] + src[(d+64)%128, r]*sin[d, c0+r].
        dst/src are [128, 512]; sin carries the sign for the lower half."""
        cols = slice(c0, c0 + 512)
        qbf = rope.tile([128, 512], bf16, tag="qbf")
        nc.scalar.copy(qbf, src_ps)
        qsw = rope.tile([128, 512], bf16, tag="qsw")
        nc.sync.dma_start(out=qsw[0:64, :], in_=qbf[64:128, :])
        nc.sync.dma_start(out=qsw[64:128, :], in_=qbf[0:64, :])
        t1 = rope.tile([128, 512], bf16, tag="t1")
        nc.vector.tensor_mul(t1, qbf, cos_sb[:, cols])
        t2 = rope.tile([128, 512], bf16, tag="t2")
        nc.vector.tensor_mul(t2, qsw, sin_sb[:, cols])
        nc.vector.tensor_add(dst, t1, t2)

    # ---- per-head stream: projections + RoPE + attention ----
    for h in range(NH):
        # stream this head's weight columns (HWDGE f32 load + one ACT cast)
        whs = []
        for w_dram, wtag in ((wq, "wq_h"), (wk, "wk_h"), (wv, "wv_h")):
            w_f = wstr.tile([128, 16, 128], f32, tag="w_f", bufs=3)
            nc.sync.dma_start(
                out=w_f,
                in_=w_dram[:, h * 128:(h + 1) * 128]
                .rearrange("(kt p) f -> p kt f", p=128),
            )
            w_b = wstr.tile([128, 16, 128], bf16, tag=wtag)
            nc.scalar.copy(w_b, w_f)
            whs.append(w_b)
        wq_h, wk_h, wv_h = whs

        # kT for the full window (3 chunks of 512 rows), with RoPE
        kr_h = kvp.tile([128, NWT, 128], bf16, tag="kr_h")
        for rb in range(3):
            ps = ps_b.tile([128, 512], f32, tag="b", name=f"kp{h}_{rb}")
            for kt in range(16):
                nc.tensor.matmul(
                    ps,
                    lhsT=wk_h[:, kt, :],
                    rhs=hsT[:, kt, rb * 512:(rb + 1) * 512],
                    start=(kt == 0),
                    stop=(kt == 15),
                )
            dst = kr_h[:, rb * 4:(rb + 1) * 4, :].rearrange("p a b -> p (a b)")
            rope_pair(dst, ps, rb * 512)

        # qT for the core's own rows (= window rows [1024, 1536)), with RoPE
        qr_h = kvp.tile([128, NQT, 128], bf16, tag="qr_h")
        ps = ps_b.tile([128, 512], f32, tag="b", name=f"qp{h}")
        for kt in range(16):
            nc.tensor.matmul(
                ps,
                lhsT=wq_h[:, kt, :],
                rhs=hsT[:, kt, 1024:1536],
                start=(kt == 0),
                stop=(kt == 15),
            )
        rope_pair(qr_h.rearrange("p a b -> p (a b)"), ps, 1024)

        # vT for the window, then PE-transpose to natural V (+ ones column)
        v_h = kvp.tile([128, NWT, 130], bf16, tag="v_h")
        nc.vector.memset(v_h[:, :, 128:130], 0.0)
        nc.vector.memset(v_h[:, :, 128:129], 1.0)
        for rb in range(3):
            ps = ps_b.tile([128, 512], f32, tag="b", name=f"vp{h}_{rb}")
            for kt in range(16):
                nc.tensor.matmul(
                    ps,
                    lhsT=wv_h[:, kt, :],
                    rhs=hsT[:, kt, rb * 512:(rb + 1) * 512],
                    start=(kt == 0),
                    stop=(kt == 15),
                )
            vt_bf = rope.tile([128, 512], bf16, tag="vt_bf")
            nc.scalar.copy(vt_bf, ps)
            for j in range(4):
                tp = ps_s.tile([128, 128], bf16, tag="s", name=f"vt{h}_{rb}_{j}")
                nc.tensor.transpose(tp, vt_bf[:, j * 128:(j + 1) * 128], idb_sb)
                nc.scalar.copy(v_h[:, rb * 4 + j, 0:128], tp)

        # attention for the 4 local q tiles
        for lq in range(NQT):
            o_ps = ps_s.tile([128, 132], f32, tag="s", name=f"o{h}_{lq}")
            p_sb = attn.tile([128, NKT, 128], bf16, tag="p_sb")
            for t in range(NKT):
                st_ps = ps_s.tile([128, 128], f32, tag="s", name=f"st{h}_{lq}_{t}")
                nc.tensor.matmul(
                    st_ps,
                    lhsT=kr_h[:, lq + t, :],
                    rhs=qr_h[:, lq, :],
                    start=True,
                    stop=True,
                )
                if t == NKT - 1:
                    nc.vector.tensor_add(st_ps, st_ps, maskd_sb)
                elif t == 0:
                    nc.vector.tensor_add(st_ps, st_ps, maskl_sb)
                nc.scalar.activation(
                    p_sb[:, t, :], st_ps, AF.Exp,
                    bias=tsel_sb[:, lq, t:t + 1], scale=SCALE,
                )
            for t in range(NKT):
                nc.tensor.matmul(
                    o_ps[:, 0:129],
                    lhsT=p_sb[:, t, :],
                    rhs=v_h[:, lq + t, 0:129],
                    start=(t == 0),
                    stop=(t == NKT - 1),
                )
            rinv = attn.tile([128, 1], f32, tag="rinv")
            nc.vector.reciprocal(rinv, o_ps[:, 128:129])
            ao = attn.tile([128, 128], bf16, tag="ao")
            nc.vector.tensor_scalar_mul(ao, o_ps[:, 0:128], rinv)
            tp = ps_s.tile([128, 128], bf16, tag="s", name=f"aot{h}_{lq}")
            nc.tensor.transpose(tp, ao, idb_sb)
            nc.scalar.copy(ot_sb[:, h, lq, :], tp)

    # ---- o_proj: out[rows, :] = ot.T @ wo, streaming wo (bf16 bounce).
    # Each streamed wo tile is reused for a pair of q-tiles to halve traffic.
    for lqp in range(NQT // 2):
        for nb in range(4):
            pss = [
                ps_b.tile([128, 512], f32, tag="b", name=f"op{lqp}_{nb}_{i}")
                for i in range(2)
            ]
            for kt in range(16):
                wos = phc.tile([128, 512], bf16, tag="wos", bufs=6)
                nc.sync.dma_start(
                    out=wos,
                    in_=wo_bf[kt * 128:(kt + 1) * 128, nb * 512:(nb + 1) * 512],
                )
                for i in range(2):
                    nc.tensor.matmul(
                        pss[i], lhsT=ot_sb[:, kt, lqp * 2 + i, :], rhs=wos,
                        start=(kt == 0), stop=(kt == 15),
                    )
            for i in range(2):
                lq = lqp * 2 + i
                ob = phc.tile([128, 512], f32, tag="ob", bufs=3)
                nc.scalar.copy(ob, pss[i])
                nc.sync.dma_start(
                    out=out[lq, :, nb * 512:(nb + 1) * 512], in_=ob
                )

    ctx.close()


def _build():
    import concourse.bacc as bacc
    import concourse.tile as tile
    from concourse import mybir

    f32 = mybir.dt.float32
    bf16 = mybir.dt.bfloat16

    nc = bacc.Bacc("TRN2", target_bir_lowering=False, debug=False,
                   num_devices=NCORES)
    aps = [
        nc.dram_tensor("hsw", [WROWS, H], f32, kind="ExternalInput").ap(),
        nc.dram_tensor("wq", [H, H], f32, kind="ExternalInput").ap(),
        nc.dram_tensor("wk", [H, H], f32, kind="ExternalInput").ap(),
        nc.dram_tensor("wv", [H, H], f32, kind="ExternalInput").ap(),
        nc.dram_tensor("wo", [H, H], f32, kind="ExternalInput").ap(),
        nc.dram_tensor("cosw", [D, WROWS], bf16, kind="ExternalInput").ap(),
        nc.dram_tensor("sinw", [D, WROWS], bf16, kind="ExternalInput").ap(),
        nc.dram_tensor("tsel", [D, NQT, NKT], f32, kind="ExternalInput").ap(),
        nc.dram_tensor("maskl", [128, 128], f32, kind="ExternalInput").ap(),
        nc.dram_tensor("maskd", [128, 128], f32, kind="ExternalInput").ap(),
        nc.dram_tensor("idf", [128, 128], f32, kind="ExternalInput").ap(),
        nc.dram_tensor("idb", [128, 128], bf16, kind="ExternalInput").ap(),
        nc.dram_tensor("out", [NQT, 128, H], f32, kind="ExternalOutput").ap(),
    ]
    with tile.TileContext(nc) as tc:
        _trace(tc, aps)
    nc.compile()
    return nc


def _host_constants():
    import ml_dtypes

    inv = 1.0 / (ROPE_THETA ** (np.arange(0, D, 2, dtype=np.float64) / D))
    ii = np.arange(128)
    # masks for S^T [j, i] tiles; valid -> 0, invalid -> NEG
    maskl = np.where(ii[:, None] > ii[None, :], 0.0, NEG).astype(np.float32)
    maskd = np.where(ii[:, None] <= ii[None, :], 0.0, NEG).astype(np.float32)
    idf = np.eye(128, dtype=np.float32)
    idb = np.eye(128).astype(ml_dtypes.bfloat16)

    cos_list, sin_list, tsel_list = [], [], []
    for c in range(NCORES):
        # window rows are global positions [512c - 1024, 512c + 512)
        pos = np.arange(c * QROWS - WIN, c * QROWS + QROWS, dtype=np.float64)
        pos = np.maximum(pos, 0.0)         # pad rows: value irrelevant (masked)
        ang = inv[:, None] * pos[None, :]  # [64, WROWS]
        cos_list.append(np.concatenate([np.cos(ang), np.cos(ang)], 0)
                        .astype(ml_dtypes.bfloat16))
        sin_list.append(np.concatenate([-np.sin(ang), np.sin(ang)], 0)
                        .astype(ml_dtypes.bfloat16))
        # tsel[lq, t] = 0 if local k-tile lq+t is a real (non-pad) tile else NEG
        ts = np.zeros((NQT, NKT), np.float32)
        for lq in range(NQT):
            for t in range(NKT):
                gkt = (c * QROWS - WIN) // 128 + lq + t
                if gkt < 0:
                    ts[lq, t] = NEG
        tsel_list.append(np.broadcast_to(ts, (128, NQT, NKT)).copy())
    return cos_list, sin_list, tsel_list, maskl, maskd, idf, idb


def _get_state():
    if "nc" not in _CACHE:
        _CACHE["nc"] = _build()
        _CACHE["consts"] = _host_constants()
    return _CACHE["nc"], _CACHE["consts"]


def _in_maps(hidden_states, wq, wk, wv, wo, consts):
    hs = np.ascontiguousarray(np.asarray(hidden_states, np.float32).reshape(L, H))
    wq = np.ascontiguousarray(np.asarray(wq, np.float32))
    wk = np.ascontiguousarray(np.asarray(wk, np.float32))
    wv = np.ascontiguousarray(np.asarray(wv, np.float32))
    wo = np.ascontiguousarray(np.asarray(wo, np.float32))
    cos_list, sin_list, tsel_list, maskl, maskd, idf, idb = consts
    maps = []
    for c in range(NCORES):
        lo = c * QROWS - WIN
        hsw = np.zeros((WROWS, H), np.float32)
        src_lo = max(lo, 0)
        hsw[src_lo - lo:] = hs[src_lo:c * QROWS + QROWS]
        maps.append({
            "hsw": hsw,
            "wq": wq,
            "wk": wk,
            "wv": wv,
            "wo": wo,
            "cosw": cos_list[c],
            "sinw": sin_list[c],
            "tsel": tsel_list[c],
            "maskl": maskl,
            "maskd": maskd,
            "idf": idf,
            "idb": idb,
        })
    return maps


def _gather(results):
    full = np.empty((L, H), np.float32)
    for c in range(NCORES):
        full[c * QROWS:(c + 1) * QROWS] = results[c]["out"].reshape(QROWS, H)
    return full.reshape(1, L, H)


class _Runner:
    """Persistent jitted shard_map executable over the 8 axon cores.

    Mirrors bass2jax.run_bass_via_pjrt's multi-core path, but builds the
    jitted callable once (so repeat kernel() calls skip retracing) and
    skips output-buffer donation (this kernel writes every output element,
    so the pre-zeroed-output contract is not needed).
    """

    def __init__(self, nc):
        import jax
        from jax.sharding import Mesh, PartitionSpec
        from jax.experimental.shard_map import shard_map
        from concourse import mybir
        from concourse import bass2jax

        bass2jax.install_neuronx_cc_hook()

        partition_name = (
            nc.partition_id_tensor.name if nc.partition_id_tensor else None
        )
        in_names, out_names, out_avals, zero_outs = [], [], [], []
        for alloc in nc.m.functions[0].allocations:
            if not isinstance(alloc, mybir.MemoryLocationSet):
                continue
            name = alloc.memorylocations[0].name
            if alloc.kind == "ExternalInput":
                if name != partition_name:
                    in_names.append(name)
            elif alloc.kind == "ExternalOutput":
                out_names.append(name)
                shape = tuple(alloc.tensor_shape)
                dtype = mybir.dt.np(alloc.dtype)
                out_avals.append(jax.core.ShapedArray(shape, dtype))
                zero_outs.append(np.zeros(shape, dtype))
        self.n_params = len(in_names)
        self.in_names = list(in_names)
        self.out_names = out_names
        all_names = in_names + out_names
        if partition_name is not None:
            all_names = all_names + [partition_name]

        def _body(*args):
            operands = list(args)
            if partition_name is not None:
                operands.append(bass2jax.partition_id_tensor())
            outs = bass2jax._bass_exec_p.bind(
                *operands,
                out_avals=tuple(out_avals),
                in_names=tuple(all_names),
                out_names=tuple(out_names),
                lowering_input_output_aliases=(),
                sim_require_finite=True,
                sim_require_nnan=True,
                nc=nc,
            )
            return tuple(outs)

        devices = jax.devices()[:NCORES]
        assert len(devices) == NCORES
        self.mesh = Mesh(np.asarray(devices), ("core",))
        in_specs = (PartitionSpec("core"),) * (self.n_params + len(out_names))
        out_specs = (PartitionSpec("core"),) * len(out_names)
        self.sharded = jax.jit(
            shard_map(_body, mesh=self.mesh, in_specs=in_specs,
                      out_specs=out_specs, check_rep=False),
            keep_unused=True,
        )
        self.out_avals = out_avals
        self.concat_zeros = [
            np.zeros((NCORES * z.shape[0], *z.shape[1:]), z.dtype)
            for z in zero_outs
        ]
        self._dev_args = None

    def pack(self, maps):
        return [
            np.concatenate([np.asarray(maps[c][n]) for c in range(NCORES)], axis=0)
            for n in self.in_names
        ]

    def run(self, maps):
        import jax

        concat_in = self.pack(maps)
        out_arrs = self.sharded(*concat_in, *self.concat_zeros)
        return [
            {
                n: np.asarray(out_arrs[i]).reshape(
                    NCORES, *self.out_avals[i].shape)[c]
                for i, n in enumerate(self.out_names)
            }
            for c in range(NCORES)
        ]

    def bench(self, maps, iters=10):
        """Time repeated executions with inputs resident on device."""
        import time

        import jax

        args = [jax.device_put(a) for a in self.pack(maps)]
        args += [jax.device_put(z) for z in self.concat_zeros]
        out = self.sharded(*args)  # warm
        jax.block_until_ready(out)
        t0 = time.perf_counter()
        for _ in range(iters):
            out = self.sharded(*args)
        jax.block_until_ready(out)
        return (time.perf_counter() - t0) / iters


def _get_runner():
    nc, consts = _get_state()
    if "runner" not in _CACHE:
        _CACHE["runner"] = _Runner(nc)
    return _CACHE["runner"], consts


def kernel(hidden_states, wq, wk, wv, wo):
    runner, consts = _get_runner()
    maps = _in_maps(hidden_states, wq, wk, wv, wo, consts)
    return _gather(runner.run(maps))


def bench(hidden_states, wq, wk, wv, wo, iters=10):
    runner, consts = _get_runner()
    maps = _in_maps(hidden_states, wq, wk, wv, wo, consts)
    return runner.bench(maps, iters=iters)
